# revision 46
# baseline (speedup 1.0000x reference)
"""Trainium2 Bass kernel for NodeGraphTransformerLayer (GNN message passing).

Strategy (8 NeuronCores, SPMD single program):
  - Node space padded to NPAD = 8 * NPC (NPC = nwin*128). Core c owns nodes
    [c*NPC, (c+1)*NPC) and ALL edges whose dst falls there, sorted by dst
    window. No cross-core reduction: each core computes its rows fully.
  - Host prep: per core, edges partitioned into 128-node dst windows; within
    a window, edges sorted into A (src < NPAD/2) then B (src >= NPAD/2)
    segments, each padded to a 128 multiple (per-window static counts, maxed
    over cores). Padding edges carry dst = -1 so one-hot segment sums drop
    them. Edge spatial rows (transposed, f16), dst values (f16) and wrapped
    int16 gather indices are staged per slot.
  - Device, phase 1: KV table [NPAD, 512] f16 = h @ [Wk|Wv] + b, replicated
    on every core (feeds gathers from the two half-tables).
  - Phase 2 (per window): one dma_gather per A/B segment pulls KV[src] rows
    for the whole window; per 4-block group: one-hot matmuls expand Q[dst],
    ACT evicts them to f16, DVE computes K*Q, GPSIMD reduces per-head
    scores, spatial scores come from small matmuls, exp on ACT (table loaded
    once), V*score messages accumulate into a PSUM wV/z accumulator via
    one-hot matmuls. Result evicted to f16 SBUF (wvbuf/zrecb).
  - Phase 3 (batches of 7 windows): gate/Wo/FFN matmuls in f16, activations
    batched per function set (sigmoid -> sqrt -> gelu -> sqrt per batch) to
    avoid ACT table thrash; biases folded into matmul chains via ones-row
    matmuls; layernorm+batchnorm folded into per-channel scale/shift.
"""

import sys
from contextlib import ExitStack

import numpy as np

sys.path.insert(0, "/opt/trn_rl_repo")

import concourse.bass as bass
import concourse.tile as tile
from concourse import bacc, mybir
from concourse.bass_utils import run_bass_kernel_spmd

F32 = mybir.dt.float32
F16 = mybir.dt.float16
I32 = mybir.dt.int32
I16 = mybir.dt.int16
AF = mybir.ActivationFunctionType
ALU = mybir.AluOpType
AX = mybir.AxisListType

N, E, DIN, DOUT, H, HD, FF = 50000, 800000, 256, 256, 8, 32, 1024
NCORES = 8
NWIN = 49
SCALE = float(np.sqrt(DOUT // H))
EPS_LN = 1e-5
EPS_BN = 1e-5
BW = 7  # phase-3 batch width (windows per batch)
DEBUG_HAT = False
DEBUG_KV = -1
DEBUG_P3 = False


class Cfg:
    def __init__(self, nwin, bA, bB, ncores=NCORES):
        self.ncores = ncores
        self.nwin = nwin
        self.bA = tuple(bA)           # per-window A-segment blocks
        self.bB = tuple(bB)           # per-window B-segment blocks
        self.nb = tuple(a + b for a, b in zip(self.bA, self.bB))
        self.nbmax = max(self.nb)
        self.boff = tuple(int(x) for x in
                          np.concatenate([[0], np.cumsum(self.nb)]))
        self.NB = self.boff[-1]       # total blocks per core
        self.EP = self.NB * 128       # edge slots per core
        self.npc = nwin * 128
        self.npad = self.npc * ncores
        self.nhalf = self.npad // 2


def build(cfg: Cfg):
    nc = bacc.Bacc("TRN2", target_bir_lowering=False, debug=False,
                   num_devices=cfg.ncores)

    def inp(name, shape, dtype=F32):
        return nc.dram_tensor(name, list(shape), dtype, kind="ExternalInput")

    h_T = inp("h_T", [256, cfg.npad], F16)
    hsT = inp("hsT", [256, cfg.npc], F16)
    hwin_d = inp("hwin", [cfg.npc, 256], F16)     # h slice + bo
    spT_d = inp("spT", [256, cfg.EP], F16)
    dstseq = inp("dstseq", [1, cfg.EP], F16)
    dstcol_d = inp("dstcol", [128, cfg.NB], F32)
    gidx_d = inp("gidx", [128, cfg.NB * 8], I16)  # wrapped+replicated idxs
    Wkv = inp("Wkv", [256, 512], F16)
    bkv_row = inp("bkv_row", [1, 512], F16)
    Wq = inp("Wq", [256, 256], F16)
    bq_row = inp("bq_row", [1, 256], F16)
    Wsp = inp("Wsp", [256, 8], F16)
    bsp_row = inp("bsp_row", [1, 8], F16)
    Wg = inp("Wg", [512, 256], F16)
    bg_row = inp("bg_row", [1, 256], F16)
    Wo = inp("Wo", [256, 256], F16)
    W1 = inp("W1", [256, 1024], F16)
    b1_row = inp("b1_row", [1, 1024], F16)
    W2 = inp("W2", [1024, 256], F16)
    b2_row = inp("b2_row", [1, 256], F16)
    cs1 = inp("cs1", [128, 256], F16)
    cb1 = inp("cb1", [128, 256], F16)
    cs2 = inp("cs2", [128, 256], F16)
    cb2 = inp("cb2", [128, 256], F16)
    iota_r = inp("iota_r", [128, 128], F16)
    iota_c = inp("iota_c", [128, 1], F32)
    ident = inp("ident", [128, 128], F16)
    ehead = inp("ehead", [8, 256], F16)
    ones_row = inp("ones_row", [1, 512], F16)
    out_d = nc.dram_tensor("out", [cfg.npc, 256], F16, kind="ExternalOutput")
    dbg_d = nc.dram_tensor("dbg", [cfg.npc, 256], F16, kind="ExternalOutput") \
        if DEBUG_HAT else None
    dbgkv_d = nc.dram_tensor("dbgkv", [128, 24 * 512], F16, kind="ExternalOutput") \
        if DEBUG_KV >= 0 else None
    dbgm_d = nc.dram_tensor("dbgm", [128, 24 * 264], F16, kind="ExternalOutput") \
        if DEBUG_KV >= 0 else None
    dbgo_d = nc.dram_tensor("dbgo", [128, 24 * 128], F16, kind="ExternalOutput") \
        if DEBUG_KV >= 0 else None
    dbgx_d = nc.dram_tensor("dbgx", [cfg.npc, 256], F16, kind="ExternalOutput") \
        if DEBUG_P3 else None
    dbgx2_d = nc.dram_tensor("dbgx2", [cfg.npc, 256], F16, kind="ExternalOutput") \
        if DEBUG_P3 else None
    dbgx3_d = nc.dram_tensor("dbgx3", [cfg.npc, 256], F16, kind="ExternalOutput") \
        if DEBUG_P3 else None
    dbgx1_d = nc.dram_tensor("dbgx1", [cfg.npc, 256], F16, kind="ExternalOutput") \
        if DEBUG_P3 else None
    dbgw_d = nc.dram_tensor("dbgw", [128, 384], F16, kind="ExternalOutput") \
        if DEBUG_KV >= 0 else None
    kvtA = nc.dram_tensor("kv_tableA", [cfg.nhalf, 512], F16)
    kvtB = nc.dram_tensor("kv_tableB", [cfg.nhalf, 512], F16)

    with tile.TileContext(nc) as tc, ExitStack() as ctx:
        const = ctx.enter_context(tc.tile_pool(name="const", bufs=1))

        def ctile(src, shape, dtype=F16, tag=None, rearr=None):
            t = const.tile(list(shape), dtype, tag=tag or src.name)
            s = src[:]
            if rearr is not None:
                s = s.rearrange(rearr[0], **rearr[1])
            nc.sync.dma_start(t[:], s)
            return t

        kvw = ctile(Wkv, [128, 2, 512], rearr=("(s p) n -> p s n", dict(p=128)))
        qw = ctile(Wq, [128, 2, 256], rearr=("(s p) n -> p s n", dict(p=128)))
        spw = ctile(Wsp, [128, 2, 8], rearr=("(s p) n -> p s n", dict(p=128)))
        wgw = ctile(Wg, [128, 4, 256], rearr=("(s p) n -> p s n", dict(p=128)))
        wow = ctile(Wo, [128, 2, 256], rearr=("(s p) n -> p s n", dict(p=128)))
        w1w = ctile(W1, [128, 2, 1024], rearr=("(s p) n -> p s n", dict(p=128)))
        w2w = ctile(W2, [128, 8, 256], rearr=("(s p) n -> p s n", dict(p=128)))
        bkvr = ctile(bkv_row, [1, 512])
        bqr = ctile(bq_row, [1, 256])
        bspr = ctile(bsp_row, [1, 8])
        bgr = ctile(bg_row, [1, 256])
        b1r = ctile(b1_row, [1, 1024])
        b2r = ctile(b2_row, [1, 256])
        cs1t = ctile(cs1, [128, 256]); cb1t = ctile(cb1, [128, 256])
        cs2t = ctile(cs2, [128, 256]); cb2t = ctile(cb2, [128, 256])
        iotar = ctile(iota_r, [128, 128]); iotac = ctile(iota_c, [128, 1], dtype=F32)
        idt = ctile(ident, [128, 128]); eh = ctile(ehead, [8, 256])
        onesr = ctile(ones_row, [1, 512])
        dstc_sb = ctile(dstcol_d, [128, cfg.NB], dtype=F32)
        # persistent phase-2 -> phase-3 buffer: hat = wV / (z + eps)
        hatbuf = const.tile([128, cfg.nwin, 256], F16, tag="hatbuf")
        zcol = const.tile([128, 1], F32, tag="zcol")
        nc.gpsimd.memset(zcol[:], 0.0)
        epscol = const.tile([128, 1], F32, tag="epscol")
        nc.gpsimd.memset(epscol[:], EPS_LN)
        nc.const_aps.aps[(F32, 0.0)] = zcol[:]
        nc.const_aps.aps[(F32, EPS_LN)] = epscol[:]

        # ---------------- phase 1: KV table (A half then B half) ----------------
        ST = 512
        assert cfg.nhalf % ST == 0
        with tc.tile_pool(name="p1", bufs=2) as p1, \
             tc.tile_pool(name="p1ps", bufs=2, space="PSUM") as p1ps, \
             tc.tile_pool(name="p1o", bufs=3) as p1o:
            for hf, kvt in ((0, kvtA), (1, kvtB)):
                for s in range(cfg.nhalf // ST):
                    n0 = hf * cfg.nhalf + s * ST
                    ht = p1.tile([128, 2, ST], F16, tag="ht")
                    nc.sync.dma_start(
                        ht[:],
                        h_T[0:256, n0:n0 + ST].rearrange("(s p) e -> p s e", p=128))
                    ot = p1o.tile([128, ST // 128, 512], F16, tag="kvo")
                    for t in range(ST // 128):
                        ps = p1ps.tile([128, 512], F32, tag="kvps")
                        nc.tensor.matmul(ps[:], lhsT=ht[:, 0, t * 128:(t + 1) * 128],
                                         rhs=kvw[:, 0, :], start=True, stop=False)
                        nc.tensor.matmul(ps[:], lhsT=ht[:, 1, t * 128:(t + 1) * 128],
                                         rhs=kvw[:, 1, :], start=False, stop=False)
                        nc.tensor.matmul(ps[:], lhsT=onesr[0:1, 0:128],
                                         rhs=bkvr[0:1, :], start=False, stop=True)
                        nc.scalar.activation(out=ot[:, t, :], in_=ps[:],
                                             func=AF.Copy)
                    nc.sync.dma_start(
                        kvt[s * ST:(s + 1) * ST, :]
                            .rearrange("(t p) c -> p t c", p=128),
                        ot[:])

        # ---------------- phase 2: attention ----------------
        p2 = ctx.enter_context(tc.tile_pool(name="p2", bufs=2))
        kvp = ctx.enter_context(tc.tile_pool(name="kvgp", bufs=2))
        p2s = ctx.enter_context(tc.tile_pool(name="p2s", bufs=2))
        ph2 = ExitStack()
        ps_q = ph2.enter_context(tc.tile_pool(name="ps_q", bufs=1, space="PSUM"))
        ps_wv = ph2.enter_context(tc.tile_pool(name="ps_wv", bufs=2, space="PSUM"))
        ps_sp = ph2.enter_context(tc.tile_pool(name="ps_sp", bufs=2, space="PSUM"))
        ps_qe = ph2.enter_context(tc.tile_pool(name="ps_qe", bufs=1, space="PSUM"))

        for w in range(cfg.nwin):
            nb = cfg.nb[w]
            bA = cfg.bA[w]
            ne = nb * 128
            base = cfg.boff[w]
            eoff = base * 128
            idxt = p2.tile([128, cfg.nbmax * 8], I16, tag="idxt")
            nc.sync.dma_start(idxt[:, 0:nb * 8],
                              gidx_d[:, base * 8:(base + nb) * 8])
            spt = p2.tile([128, 2, cfg.nbmax * 128], F16, tag="spt")
            nc.sync.dma_start(
                spt[:, 0:2, 0:ne],
                spT_d[0:256, eoff:eoff + ne].rearrange("(s p) e -> p s e", p=128))
            dstb = p2.tile([128, cfg.nbmax * 128], F16, tag="dstb")
            nc.sync.dma_start(
                dstb[:, 0:ne],
                dstseq[0:1, eoff:eoff + ne].partition_broadcast(128))
            kvg = kvp.tile([128, cfg.nbmax, 512], F16, tag="kvg")
            if bA > 0:
                nc.gpsimd.dma_gather(
                    kvg[:, 0:bA, :], kvtA[:], idxt[:, 0:bA * 8],
                    bA * 128, bA * 128, 512, single_packet=False)
            if nb - bA > 0:
                nc.gpsimd.dma_gather(
                    kvg[:, bA:nb, :], kvtB[:],
                    idxt[:, bA * 8:nb * 8],
                    (nb - bA) * 128, (nb - bA) * 128, 512, single_packet=False)
            if DEBUG_KV == w:
                nc.sync.dma_start(dbgkv_d[:, 0:nb * 512],
                                  kvg[:, 0:nb, :].rearrange("p b c -> p (b c)"))
            # per-window Q (scaled): [128n, 256c] f16
            hst = p2.tile([128, 2, 128], F16, tag="hst")
            nc.sync.dma_start(hst[:, 0, :], hsT[0:128, w * 128:(w + 1) * 128])
            nc.sync.dma_start(hst[:, 1, :], hsT[128:256, w * 128:(w + 1) * 128])
            qps = ps_q.tile([128, 256], F32, tag="qps")
            nc.tensor.matmul(qps[:], lhsT=hst[:, 0, :], rhs=qw[:, 0, :],
                             start=True, stop=False)
            nc.tensor.matmul(qps[:], lhsT=hst[:, 1, :], rhs=qw[:, 1, :],
                             start=False, stop=False)
            nc.tensor.matmul(qps[:], lhsT=onesr[0:1, 0:128],
                             rhs=bqr[0:1, :], start=False, stop=True)
            qwin = p2s.tile([128, 256], F16, tag="qwin")
            nc.scalar.activation(out=qwin[:], in_=qps[:], func=AF.Copy)

            wv = ps_wv.tile([128, 384], F32, tag="wv")
            for g0 in range(0, nb, 4):
                gs = min(4, nb - g0)
                sp8g = ps_sp.tile([128, 32], F32, tag="sp8")
                for j in range(gs):
                    b = g0 + j
                    sl = sp8g[:, j * 8:(j + 1) * 8]
                    nc.tensor.matmul(sl, lhsT=spt[:, 0, b * 128:(b + 1) * 128],
                                     rhs=spw[:, 0, :], start=(j == 0), stop=False,
                                     skip_group_check=True)
                    nc.tensor.matmul(sl, lhsT=spt[:, 1, b * 128:(b + 1) * 128],
                                     rhs=spw[:, 1, :], start=False, stop=False,
                                     skip_group_check=True)
                    nc.tensor.matmul(sl, lhsT=onesr[0:1, 0:128],
                                     rhs=bspr[0:1, :], start=False,
                                     stop=(j == gs - 1), skip_group_check=True)
                ohT4 = p2.tile([128, 4, 128], F16, tag="ohT")
                nc.vector.tensor_scalar(
                    out=ohT4[:, 0:gs, :],
                    in0=dstb[:, g0 * 128:(g0 + gs) * 128].rearrange(
                        "p (g n) -> p g n", n=128),
                    scalar1=iotac[:, 0:1], scalar2=None, op0=ALU.is_equal)
                oh4 = p2.tile([128, 4, 128], F16, tag="oh")
                for j in range(gs):
                    nc.vector.tensor_scalar(
                        out=oh4[:, j, :], in0=iotar[:],
                        scalar1=dstc_sb[:, base + g0 + j:base + g0 + j + 1],
                        scalar2=None, op0=ALU.is_equal)
                qe = ps_qe.tile([128, 4, 256], F32, tag="qe")
                for j in range(gs):
                    nc.tensor.matmul(qe[:, j, :], lhsT=ohT4[:, j, :],
                                     rhs=qwin[:], start=True, stop=True,
                                     skip_group_check=True)
                qes = p2.tile([128, 4, 256], F16, tag="qes")
                nc.scalar.activation(out=qes[:, 0:gs, :], in_=qe[:, 0:gs, :],
                                     func=AF.Copy)
                tsb = p2.tile([128, 4, 256], F16, tag="tsb")
                nc.vector.tensor_tensor(out=tsb[:, 0:gs, :],
                                        in0=kvg[:, g0:g0 + gs, 0:256],
                                        in1=qes[:, 0:gs, :], op=ALU.mult)
                # fold d=32 -> 16 -> 8 with 2x tensor_tensor adds, then a
                # smaller (1x-only) tensor_reduce
                th = p2.tile([128, 4, 8, 16], F16, tag="th")
                tq = p2.tile([128, 4, 8, 8], F16, tag="tq")
                s84 = p2.tile([128, 4, 8], F16, tag="s84")
                tsbv = tsb[:, 0:gs, :].rearrange("p g (h d) -> p g h d", d=32)
                nc.vector.tensor_tensor(out=th[:, 0:gs, :, :],
                                        in0=tsbv[:, :, :, 0:16],
                                        in1=tsbv[:, :, :, 16:32], op=ALU.add)
                nc.vector.tensor_tensor(out=tq[:, 0:gs, :, :],
                                        in0=th[:, 0:gs, :, 0:8],
                                        in1=th[:, 0:gs, :, 8:16], op=ALU.add)
                with nc.allow_low_precision(reason="f16 scores, tol 2e-2"):
                    nc.vector.tensor_reduce(
                        out=s84[:, 0:gs, :], in_=tq[:, 0:gs, :, :],
                        axis=AX.X, op=ALU.add)
                mext4 = p2.tile([128, 4, 264], F16, tag="mext")
                sst4 = p2.tile([128, 4, 8], F16, tag="sst4")
                nc.vector.tensor_tensor(
                    out=sst4[:, 0:gs, :], in0=s84[:, 0:gs, :],
                    in1=sp8g[:].rearrange("p (g h) -> p g h", h=8)[:, 0:gs, :],
                    op=ALU.add)
                nc.vector.tensor_scalar(out=sst4[:, 0:gs, :], in0=sst4[:, 0:gs, :],
                                        scalar1=5.0, scalar2=-5.0,
                                        op0=ALU.min, op1=ALU.max)
                nc.scalar.activation(out=mext4[:, 0:gs, 256:264],
                                     in_=sst4[:, 0:gs, :], func=AF.Exp)
                # V channels are head-interleaved (c' = d*8 + h) so the score
                # broadcast lands on a stride-1 innermost axis (2x DVE mode)
                nc.vector.tensor_tensor(
                    out=mext4[:, 0:gs, 0:256].rearrange(
                        "p g (d h) -> p g d h", h=8),
                    in0=kvg[:, g0:g0 + gs, 256:512].rearrange(
                        "p g (d h) -> p g d h", h=8),
                    in1=mext4[:, 0:gs, 256:264]
                        .rearrange("p g (o h) -> p g o h", o=1)
                        .to_broadcast([128, gs, 32, 8]),
                    op=ALU.mult)
                if DEBUG_KV == w:
                    nc.sync.dma_start(
                        dbgm_d[:, g0 * 264:(g0 + gs) * 264],
                        mext4[:, 0:gs, :].rearrange("p g c -> p (g c)"))
                    nc.sync.dma_start(
                        dbgo_d[:, g0 * 128:(g0 + gs) * 128],
                        oh4[:, 0:gs, :].rearrange("p g n -> p (g n)"))
                for j in range(gs):
                    b = g0 + j
                    st = b == 0
                    fin = b == nb - 1
                    nc.tensor.matmul(wv[:, 0:128], lhsT=mext4[:, j, 0:128],
                                     rhs=oh4[:, j, :], start=st, stop=False,
                                     skip_group_check=True)
                    nc.tensor.matmul(wv[:, 128:256], lhsT=mext4[:, j, 128:256],
                                     rhs=oh4[:, j, :], start=False, stop=False,
                                     skip_group_check=True)
                    nc.tensor.matmul(wv[0:8, 256:384], lhsT=mext4[:, j, 256:264],
                                     rhs=oh4[:, j, :], start=False, stop=fin,
                                     skip_group_check=True)
            # evict per-window results: hat = wV * (1 / (z + eps)) -> f16
            if DEBUG_KV == w:
                wvdbg = p2.tile([128, 384], F16, tag="wvdbg")
                nc.vector.tensor_copy(out=wvdbg[:], in_=wv[:])
                nc.sync.dma_start(dbgw_d[:], wvdbg[:])
            wvf = p2s.tile([128, 256], F16, tag="wvf")
            nc.scalar.activation(out=wvf[:], in_=wv[:, 0:256], func=AF.Copy)
            zr = p2s.tile([8, 128], F32, tag="zr")
            nc.vector.tensor_scalar(out=zr[:], in0=wv[0:8, 256:384],
                                    scalar1=1e-6, scalar2=None, op0=ALU.add)
            zrec = p2s.tile([8, 128], F16, tag="zrec")
            with nc.allow_low_precision(reason="f16 attn weights, tol 2e-2"):
                nc.vector.reciprocal(out=zrec[:], in_=zr[:])
            zrep = ps_q.tile([128, 256], F32, tag="zrep")
            nc.tensor.matmul(zrep[:, 0:128], lhsT=eh[0:8, 0:128],
                             rhs=zrec[:], start=True, stop=False)
            nc.tensor.matmul(zrep[:, 128:256], lhsT=eh[0:8, 128:256],
                             rhs=zrec[:], start=False, stop=True)
            zrs = p2s.tile([128, 256], F16, tag="zrs")
            nc.scalar.activation(out=zrs[:], in_=zrep[:], func=AF.Copy)
            nc.vector.tensor_tensor(out=hatbuf[:, w, :], in0=wvf[:],
                                    in1=zrs[:], op=ALU.mult)

        # ---------------- phase 3: gate + FFN (batched) ----------------
        ph2.close()
        p3 = ctx.enter_context(tc.tile_pool(name="p3", bufs=2))
        p3w = ctx.enter_context(tc.tile_pool(name="p3w", bufs=3))
        p3d = ctx.enter_context(tc.tile_pool(name="p3d", bufs=2))
        ps_b = ctx.enter_context(tc.tile_pool(name="ps_b", bufs=2, space="PSUM"))
        ps_gate = ctx.enter_context(
            tc.tile_pool(name="ps_gate", bufs=2, space="PSUM"))
        ps_g1 = ctx.enter_context(tc.tile_pool(name="ps_g1", bufs=2, space="PSUM"))

        nbat = (cfg.nwin + BW - 1) // BW

        def layernorm(xin, B, cst, cbt, out, sfx):
            mu = p3.tile([128, BW, 1], F32, tag="mu" + sfx)
            muh = p3.tile([128, BW, 1], F16, tag="muh" + sfx)
            vs = p3.tile([128, BW, 1], F32, tag="vs" + sfx)
            sd = p3.tile([128, BW, 1], F16, tag="sd" + sfx)
            rstd = p3.tile([128, BW, 1], F16, tag="rstd" + sfx)
            xcb = p3.tile([128, BW, 256], F16, tag="xcbs")
            sqb = p3.tile([128, BW, 256], F16, tag="sqs")
            nc.vector.tensor_reduce(out=mu[:, 0:B, :],
                                    in_=xin[:, 0:B, :], axis=AX.X, op=ALU.add)
            nc.vector.tensor_scalar_mul(out=muh[:, 0:B, :], in0=mu[:, 0:B, :],
                                        scalar1=1.0 / 256)
            nc.vector.tensor_tensor(out=xcb[:, 0:B, :], in0=xin[:, 0:B, :],
                                    in1=muh[:, 0:B, :].to_broadcast([128, B, 256]),
                                    op=ALU.subtract)
            nc.vector.tensor_tensor(out=sqb[:, 0:B, :], in0=xcb[:, 0:B, :],
                                    in1=xcb[:, 0:B, :], op=ALU.mult)
            nc.vector.tensor_reduce(out=vs[:, 0:B, :], in_=sqb[:, 0:B, :],
                                    axis=AX.X, op=ALU.add)
            nc.scalar.activation(out=sd[:, 0:B, :], in_=vs[:, 0:B, :],
                                 func=AF.Sqrt, scale=1.0 / 256, bias=EPS_LN)
            with nc.allow_low_precision(reason="f16 layernorm, tol 2e-2"):
                nc.vector.reciprocal(out=rstd[:, 0:B, :], in_=sd[:, 0:B, :])
            nc.vector.tensor_tensor(out=xcb[:, 0:B, :], in0=xcb[:, 0:B, :],
                                    in1=rstd[:, 0:B, :].to_broadcast([128, B, 256]),
                                    op=ALU.mult)
            nc.vector.tensor_tensor(
                out=out[:, 0:B, :], in0=xcb[:, 0:B, :],
                in1=cst[:].rearrange("p (o c) -> p o c", o=1)
                    .to_broadcast([128, B, 256]),
                op=ALU.mult)
            nc.vector.tensor_tensor(
                out=out[:, 0:B, :], in0=out[:, 0:B, :],
                in1=cbt[:].rearrange("p (o c) -> p o c", o=1)
                    .to_broadcast([128, B, 256]),
                op=ALU.add)

        def stage_a(bi):
            """gate -> x1 -> Wo -> residual -> LN1; returns x2in tile."""
            w0 = bi * BW
            B = min(BW, cfg.nwin - w0)
            gtsb = p3.tile([128, BW, 256], F16, tag="gtsb")
            x1b = p3.tile([128, BW, 256], F16, tag="x1b")
            xb = p3.tile([128, BW, 256], F16, tag="xb")
            x2in = p3.tile([128, BW, 256], F16, tag="x2in")
            for j in range(B):
                w = w0 + j
                hstw = p3d.tile([128, 2, 128], F16, tag="hstw")
                nc.sync.dma_start(hstw[:, 0, :], hsT[0:128, w * 128:(w + 1) * 128])
                nc.sync.dma_start(hstw[:, 1, :], hsT[128:256, w * 128:(w + 1) * 128])
                gate = ps_gate.tile([128, 256], F32, tag="psgate")
                rhs_list = [hstw[:, 0, :], hstw[:, 1, :],
                            hatbuf[:, w, 0:128], hatbuf[:, w, 128:256]]
                for ci, rr in enumerate(rhs_list):
                    for co in range(2):
                        nc.tensor.matmul(gate[:, co * 128:(co + 1) * 128],
                                         lhsT=wgw[:, ci, co * 128:(co + 1) * 128],
                                         rhs=rr, start=(ci == 0 and co == 0),
                                         stop=False, skip_group_check=True)
                nc.tensor.matmul(gate[:, 0:128], lhsT=bgr[0:1, 0:128],
                                 rhs=onesr[0:1, 0:128], start=False, stop=False,
                                 skip_group_check=True)
                nc.tensor.matmul(gate[:, 128:256], lhsT=bgr[0:1, 128:256],
                                 rhs=onesr[0:1, 0:128], start=False, stop=True,
                                 skip_group_check=True)
                nc.scalar.activation(out=gtsb[:, j, :], in_=gate[:],
                                     func=AF.Sigmoid)
            nc.vector.tensor_tensor(out=x1b[:, 0:B, :], in0=gtsb[:, 0:B, :],
                                    in1=hatbuf[:, w0:w0 + B, :], op=ALU.mult)
            if DEBUG_P3:
                nc.sync.dma_start(
                    dbgx1_d[w0 * 128:(w0 + B) * 128, :]
                        .rearrange("(j p) c -> p j c", p=128), gtsb[:, 0:B, :])
            for j in range(B):
                w = w0 + j
                yps = ps_gate.tile([128, 256], F32, tag="psgate")
                nc.tensor.matmul(yps[:], lhsT=x1b[:, j, 0:128],
                                 rhs=wow[:, 0, :], start=True, stop=False)
                nc.tensor.matmul(yps[:], lhsT=x1b[:, j, 128:256],
                                 rhs=wow[:, 1, :], start=False, stop=True)
                hwin = p3d.tile([128, 256], F16, tag="hwin")
                nc.sync.dma_start(hwin[:], hwin_d[w * 128:(w + 1) * 128, :])
                nc.vector.tensor_tensor(out=xb[:, j, :], in0=yps[:],
                                        in1=hwin[:], op=ALU.add)
            layernorm(xb, B, cs1t, cb1t, x2in, "a")
            if DEBUG_P3:
                nc.sync.dma_start(
                    dbgx_d[w0 * 128:(w0 + B) * 128, :]
                        .rearrange("(j p) c -> p j c", p=128), xb[:, 0:B, :])
                nc.sync.dma_start(
                    dbgx2_d[w0 * 128:(w0 + B) * 128, :]
                        .rearrange("(j p) c -> p j c", p=128), x2in[:, 0:B, :])
            return x2in

        def stage_b(bi, x2in):
            """FFN -> residual -> LN2 -> out DMA."""
            w0 = bi * BW
            B = min(BW, cfg.nwin - w0)
            x3b = p3.tile([128, BW, 256], F16, tag="x3b")
            xob = p3.tile([128, BW, 256], F16, tag="xob")
            for j in range(B):
                xT = ps_b.tile([128, 256], F16, tag="psbT")
                nc.tensor.matmul(xT[:, 0:128], lhsT=x2in[:, j, 0:128], rhs=idt[:],
                                 is_transpose=True, start=True, stop=False)
                nc.tensor.matmul(xT[:, 128:256], lhsT=x2in[:, j, 128:256],
                                 rhs=idt[:], is_transpose=True, start=False,
                                 stop=True)
                xTs = p3w.tile([128, 256], F16, tag="xTs")
                nc.scalar.activation(out=xTs[:], in_=xT[:], func=AF.Copy)
                g1s = p3w.tile([128, 1024], F16, tag="g1s")
                for half in range(2):
                    g1 = ps_g1.tile([128, 512], F32, tag="psg1")
                    for q in range(4):
                        ct = half * 4 + q
                        off = q * 128
                        nc.tensor.matmul(g1[:, off:off + 128],
                                         lhsT=w1w[:, 0, ct * 128:(ct + 1) * 128],
                                         rhs=xTs[:, 0:128], start=(q == 0),
                                         stop=False, skip_group_check=True)
                        nc.tensor.matmul(g1[:, off:off + 128],
                                         lhsT=w1w[:, 1, ct * 128:(ct + 1) * 128],
                                         rhs=xTs[:, 128:256], start=False,
                                         stop=False, skip_group_check=True)
                        nc.tensor.matmul(g1[:, off:off + 128],
                                         lhsT=b1r[0:1, ct * 128:(ct + 1) * 128],
                                         rhs=onesr[0:1, 0:128], start=False,
                                         stop=(q == 3), skip_group_check=True)
                    nc.scalar.activation(
                        out=g1s[:, half * 512:(half + 1) * 512],
                        in_=g1[:], func=AF.Gelu)
                x2p = ps_b.tile([128, 256], F32, tag="psb")
                for ct in range(8):
                    nc.tensor.matmul(
                        x2p[:], lhsT=g1s[:, ct * 128:(ct + 1) * 128],
                        rhs=w2w[:, ct, :], start=(ct == 0), stop=False,
                        skip_group_check=True)
                nc.tensor.matmul(x2p[:], lhsT=onesr[0:1, 0:128],
                                 rhs=b2r[0:1, :], start=False, stop=True,
                                 skip_group_check=True)
                nc.vector.tensor_tensor(out=x3b[:, j, :], in0=x2p[:],
                                        in1=x2in[:, j, :], op=ALU.add)
            if DEBUG_P3:
                nc.sync.dma_start(
                    dbgx3_d[w0 * 128:(w0 + B) * 128, :]
                        .rearrange("(j p) c -> p j c", p=128), x3b[:, 0:B, :])
            layernorm(x3b, B, cs2t, cb2t, xob, "b")
            nc.sync.dma_start(
                out_d[w0 * 128:(w0 + B) * 128, :]
                    .rearrange("(j p) c -> p j c", p=128),
                xob[:, 0:B, :])

        if DEBUG_HAT:
            for w in range(cfg.nwin):
                nc.sync.dma_start(dbg_d[w * 128:(w + 1) * 128, :],
                                  hatbuf[:, w, :])
        x2s = {}
        for bi in range(nbat + 1):
            if bi < nbat:
                x2s[bi] = stage_a(bi)
            if bi >= 1:
                stage_b(bi - 1, x2s.pop(bi - 1))

    nc.compile()
    return nc


def analyze_edges(dst, src, nwin, npc, nhalf):
    """Per-window A/B block counts, maxed over cores (uniform program)."""
    bA = np.zeros(nwin, np.int64)
    bB = np.zeros(nwin, np.int64)
    for c in range(NCORES):
        m = (dst >= c * npc) & (dst < (c + 1) * npc)
        dl = dst[m] - c * npc
        sl = src[m]
        wi = dl >> 7
        isA = sl < nhalf
        cntA = np.bincount(wi[isA], minlength=nwin)
        cntB = np.bincount(wi[~isA], minlength=nwin)
        bA = np.maximum(bA, -(-cntA // 128))
        bB = np.maximum(bB, -(-cntB // 128))
    return bA, bB


def prepare(cfg: Cfg, inputs):
    f32 = np.float32
    f16 = np.float16
    h = np.asarray(inputs["h"], f32)
    sp = np.asarray(inputs["spatial_pos"], f32)
    src = np.asarray(inputs["src"]).astype(np.int64)
    dst = np.asarray(inputs["dst"]).astype(np.int64)
    W = {k: np.asarray(inputs[k], f32) for k in
         ["Wq", "bq", "Wk", "bk", "Wv", "bv", "Wsp", "bsp", "Wo", "bo",
          "Wg", "bg", "W1", "b1", "W2", "b2", "ln1_g", "ln1_b", "ln2_g",
          "ln2_b", "bn1_g", "bn1_b", "bn2_g", "bn2_b"]}

    npc, npad, nhalf = cfg.npc, cfg.npad, cfg.nhalf
    n_real = h.shape[0]
    h_pad = np.zeros((npad, 256), f32)
    h_pad[:n_real] = h

    # V channels head-interleaved on device: c' = d*8 + h <- c = h*32 + d
    cp = np.arange(256)
    vperm = (cp % 8) * 32 + cp // 8
    Wkv = np.concatenate([W["Wk"], W["Wv"][:, vperm]], 1)
    bkv = np.concatenate([W["bk"], W["bv"][vperm]])
    Wq_s = W["Wq"] / SCALE
    bq_s = W["bq"] / SCALE
    Wsp_r = W["Wsp"].astype(np.float64).reshape(256, 8, 32).sum(-1).astype(f32)
    bsp_r = W["bsp"].astype(np.float64).reshape(8, 32).sum(-1).astype(f32)
    # reorder Wg rows: device concat layout [h(256) | h_attn(256, vperm'd)]
    # -> reference layout interleaved per head (h-head, attn-head)
    pr = np.empty(512, np.int64)
    r = np.arange(256)
    pr[:256] = (r // 32) * 64 + (r % 32)
    pr[256:] = (vperm // 32) * 64 + 32 + (vperm % 32)
    # columns (gate output channels) must match hat's vperm'd channel order
    Wg_r = W["Wg"][pr][:, vperm]
    Wo_r = W["Wo"][vperm]
    rs = 1.0 / np.sqrt(np.float32(1.0 + EPS_BN))
    cs1 = W["ln1_g"] * rs * W["bn1_g"]
    cb1 = W["ln1_b"] * rs * W["bn1_g"] + W["bn1_b"]
    cs2 = W["ln2_g"] * rs * W["bn2_g"]
    cb2 = W["ln2_b"] * rs * W["bn2_g"] + W["bn2_b"]

    rep = lambda v: np.tile(np.asarray(v, f16)[None, :], (128, 1))
    ehead = np.zeros((8, 256), f16)
    ehead[np.arange(256) % 8, np.arange(256)] = 1.0

    shared = dict(
        h_T=np.ascontiguousarray(h_pad.T).astype(f16),
        Wkv=Wkv.astype(f16), bkv_row=bkv[None, :].astype(f16),
        Wq=Wq_s.astype(f16), bq_row=bq_s[None, :].astype(f16),
        Wsp=Wsp_r.astype(f16), bsp_row=bsp_r[None, :].astype(f16),
        Wg=Wg_r.astype(f16), bg_row=W["bg"][vperm][None, :].astype(f16),
        Wo=Wo_r.astype(f16),
        W1=W["W1"].astype(f16), b1_row=W["b1"][None, :].astype(f16),
        W2=W["W2"].astype(f16), b2_row=W["b2"][None, :].astype(f16),
        cs1=rep(cs1), cb1=rep(cb1), cs2=rep(cs2), cb2=rep(cb2),
        iota_r=np.tile(np.arange(128, dtype=f16), (128, 1)),
        iota_c=np.arange(128, dtype=f32)[:, None],
        ident=np.eye(128, dtype=f16),
        ehead=ehead,
        ones_row=np.ones((1, 512), f16),
    )

    core_of = dst // npc
    in_maps = []
    for c in range(cfg.ncores):
        em = np.nonzero(core_of == c)[0]
        dl = (dst[em] - c * npc).astype(np.int64)
        sl = src[em]
        wi = dl >> 7
        # slot assignment: per window, A-edges then B-edges in their block
        # ranges [boff*128, boff*128 + bA*128) and [.. + bA*128, ..)
        isA = sl < nhalf
        slot = np.zeros(len(em), np.int64)
        for w in range(cfg.nwin):
            mw = wi == w
            base = cfg.boff[w] * 128
            ia = np.nonzero(mw & isA)[0]
            ib = np.nonzero(mw & ~isA)[0]
            slot[ia] = base + np.arange(len(ia))
            slot[ib] = base + cfg.bA[w] * 128 + np.arange(len(ib))
        srci_flat = np.zeros(cfg.EP, np.int64)
        srci_flat[slot] = sl
        # B-slot indices rebased to the second half-table
        for w in range(cfg.nwin):
            b0 = (cfg.boff[w] + cfg.bA[w]) * 128
            b1 = cfg.boff[w + 1] * 128
            srci_flat[b0:b1] = np.maximum(srci_flat[b0:b1] - nhalf, 0)
        dstf_flat = np.full(cfg.EP, -1.0, f16)
        dstf_flat[slot] = (dl - (wi << 7)).astype(f16)
        spE = np.zeros((cfg.EP, 256), f16)
        spE[slot] = sp[em]
        # wrapped int16 gather indices: idx i of block b at [i%16, b*8 + i//16]
        gidx = np.zeros((16, cfg.NB * 8), np.int16)
        si = srci_flat.reshape(cfg.NB, 128)
        for b in range(cfg.NB):
            gidx[:, b * 8:(b + 1) * 8] = si[b].reshape(8, 16).T
        gidx = np.tile(gidx, (8, 1))
        h_slice = h_pad[c * npc:(c + 1) * npc]
        m = dict(shared)
        m.update(
            hsT=np.ascontiguousarray(h_slice.T).astype(f16),
            hwin=(h_slice + W["bo"][None, :]).astype(f16),
            spT=np.ascontiguousarray(spE.T),
            dstseq=dstf_flat[None, :],
            dstcol=np.ascontiguousarray(dstf_flat.reshape(-1, 128).T).astype(f32),
            gidx=gidx,
        )
        in_maps.append(m)
    return in_maps


def make_cfg(inputs):
    dst = np.asarray(inputs["dst"]).astype(np.int64)
    src = np.asarray(inputs["src"]).astype(np.int64)
    npc = NWIN * 128
    bA, bB = analyze_edges(dst, src, NWIN, npc, npc * NCORES // 2)
    return Cfg(nwin=NWIN, bA=bA, bB=bB)


_CACHE = {}


def kernel(**inputs) -> np.ndarray:
    n_real = inputs["h"].shape[0]
    cfg = make_cfg(inputs)
    in_maps = prepare(cfg, inputs)
    key = (cfg.nwin, cfg.bA, cfg.bB)
    if key not in _CACHE:
        _CACHE[key] = build(cfg)
    nc = _CACHE[key]
    res = run_bass_kernel_spmd(nc, in_maps, list(range(cfg.ncores)))
    out = np.concatenate([res.results[c]["out"] for c in range(cfg.ncores)], 0)
    return out[:n_real].astype(np.float32)


if __name__ == "__main__":
    pass
